# revision 18
# baseline (speedup 1.0000x reference)
"""Trainium2 Bass kernel for masked (pruned-softmax) multi-head attention.

Reference computation (N=2048, DIM=768, H=12, DH=64):
    q = (x @ Wq + bq).reshape(n, H, DH)
    k = (x @ Wk + bk).reshape(n, H, DH)
    v = (x @ Wv + bv).reshape(n, H, DH)
    dots = einsum("nhd,mhd->hnm", q, k) * DH**-0.5
    attn = exp(dots) * (last_score > 0)[None, None, :]
    attn = attn / (attn.sum(-1, keepdims=True) + 1e-6)
    out  = einsum("hnm,mhd->nhd", attn, v).reshape(n, H*DH) @ Wo + bo
    return (out, last_score)

Distribution: 8 NeuronCores = 4 head-groups (3 heads each) x 2 sequence
halves (1024 queries each).  Each core:
  - receives only the *kept* keys (last_score > 0, gathered/padded on the
    host -- exploits the ~50% key sparsity of the heaviside pruning),
  - projects K/V for its 3 heads in transposed layout, streaming per
    key-chunk so compute starts while x is still loading,
  - computes scores S^T = K Q^T per key-tile, exp on ScalarE (the
    bottleneck engine), and AV with a ones-column appended to V so the
    softmax denominator falls out of the same matmul,
  - normalizes via exp(-ln(d + 1e-6)) on ScalarE (one activation table
    serves both Exp and Ln) broadcast across partitions with a K=1
    ones-matmul on the PE,
  - emits a partial output projection; the host sums the 4 head-group
    partials per sequence half and adds bo.

All matmuls run as float32r (TF32-like 1+8+11-bit operands, 1 cycle/row
vs 4 for fp32 when the moving dim is >= 256) with fp32 PSUM accumulation.
Measured end-to-end absmax error vs the fp32 reference: ~1.7e-3 of the
output scale.
"""

import numpy as np

import concourse.bacc as bacc
import concourse.tile as tile
from concourse import mybir
from concourse.bass_utils import run_bass_kernel_spmd

N_CTX, DIM, H, DH = 2048, 768, 12, 64
INNER = H * DH
SCALE = DH ** -0.5
NCORES = 8
HG = 4             # head groups
SG = 2             # sequence groups
HPC = H // HG      # heads per core = 3
QPC = N_CTX // SG  # queries per core = 1024
DT = DIM // 128    # contraction tiles = 6

F32 = mybir.dt.float32
F32R = mybir.dt.float32r

_CACHE: dict = {}


def _chunks(total, step):
    out = []
    c = 0
    while c < total:
        out.append((c, min(c + step, total)))
        c = out[-1][1]
    return out


def _chunks_min256(total, step=512):
    """Chunks of <= step, rebalanced so no chunk is narrower than 256
    (fp32r matmuls below a 256-wide moving dim run at quarter rate)."""
    out = _chunks(total, step)
    if len(out) >= 2 and out[-1][1] - out[-1][0] < 256:
        (a0, a1), (b0, b1) = out[-2], out[-1]
        mid = b1 - 256 if b1 - 256 > a0 else (a0 + b1) // 2
        out[-2], out[-1] = (a0, mid), (mid, b1)
    return out


def _build(KT: int):
    """Build + compile the per-core Bass program for KT key-tiles."""
    import concourse.bacc as _bm
    # Steer Bacc's activation-table placement to the one set containing
    # both Exp and Ln so the whole kernel needs a single ACT_TABLE_LOAD.
    # Positions (= act_func_set ids) must be preserved, so keep the full
    # list and only prune Exp/Ln from the *other* sets in the pass's view.
    _orig_tabs = _bm.get_activation_tables
    _EXP = mybir.ActivationFunctionType.Exp
    _LN = mybir.ActivationFunctionType.Ln

    def _steered(arch):
        out = {}
        for name, funcs in _orig_tabs(arch).items():
            if name != "natural_log_exp_and_others":
                funcs = {f for f in funcs if f not in (_EXP, _LN)}
            out[name] = funcs
        return out

    _bm.get_activation_tables = _steered
    try:
        return _build_inner(KT)
    finally:
        _bm.get_activation_tables = _orig_tabs


def _build_inner(KT: int):
    KK = KT * 128
    nc = bacc.Bacc("TRN2", target_bir_lowering=False, debug=False)

    xq_d = nc.dram_tensor("xq", [DIM, QPC], F32R, kind="ExternalInput")
    xkv_d = nc.dram_tensor("xkv", [DIM + 1, KK], F32R, kind="ExternalInput")
    wq_d = nc.dram_tensor("wq", [DIM, HPC * DH], F32R, kind="ExternalInput")
    wk_d = nc.dram_tensor("wk", [DIM, HPC * DH], F32R, kind="ExternalInput")
    wv_d = nc.dram_tensor("wv", [DIM + 1, 256], F32R, kind="ExternalInput")
    wo_d = nc.dram_tensor("wo", [HPC * DH, DIM], F32R, kind="ExternalInput")
    bqk_d = nc.dram_tensor("bqk", [128, 4], F32, kind="ExternalInput")
    km_d = nc.dram_tensor("km", [128, KT], F32, kind="ExternalInput")
    out_d = nc.dram_tensor("out", [QPC, DIM], F32, kind="ExternalOutput")

    kch = _chunks_min256(KK)          # key chunks (projection free dim)
    kdma = _chunks(KK, 256)           # key chunks (DMA granularity)
    ECH = DH + 1                      # 65: V columns + denominator ones col

    with tile.TileContext(nc) as tc:
        with (
            tc.tile_pool(name="const", bufs=1) as cpool,
            tc.tile_pool(name="epool", bufs=10) as epool,
            tc.tile_pool(name="nrm", bufs=2) as npool,
            tc.tile_pool(name="outsb", bufs=4) as opool,
            tc.tile_pool(name="psmm", bufs=3, space="PSUM") as ps_mm,
            tc.tile_pool(name="psop", bufs=1, space="PSUM") as ps_op,
            tc.tile_pool(name="psbc", bufs=2, space="PSUM") as ps_bc,
        ):
            # ---- constants ----
            onesc = cpool.tile([1, DH], F32, tag="onesc", name="onesc")
            nc.vector.memset(onesc[:], 1.0)
            epsc = cpool.tile([1, 1], F32, tag="epsc", name="epsc")
            nc.vector.memset(epsc[:], 1e-6)
            negc = cpool.tile([1, 1], F32, tag="negc", name="negc")
            nc.vector.memset(negc[:], -1.0)

            # ---- input DMAs, ordered for pipeline startup ----
            wv_t = cpool.tile([128, DT, 256], F32R, tag="wv", name="wv")
            nc.sync.dma_start(
                wv_t[:], wv_d.ap()[0:DIM, :].rearrange("(t p) n -> p t n", p=128))
            wv1_t = cpool.tile([1, 256], F32R, tag="wv1", name="wv1")
            nc.sync.dma_start(wv1_t[:], wv_d.ap()[DIM:DIM + 1, :])
            km_t = cpool.tile([128, KT], F32, tag="km", name="km")
            nc.sync.dma_start(km_t[:], km_d.ap())
            wk_t = cpool.tile([128, DT, HPC * DH], F32R, tag="wk", name="wk")
            nc.sync.dma_start(
                wk_t[:], wk_d.ap().rearrange("(t p) n -> p t n", p=128))
            bqk_t = cpool.tile([128, 4], F32, tag="bqk", name="bqk")
            nc.sync.dma_start(bqk_t[:], bqk_d.ap())
            xkv_t = cpool.tile([128, DT, KK], F32R, tag="xkv", name="xkv")
            xkv_re = xkv_d.ap()[0:DIM, :].rearrange("(t p) n -> p t n", p=128)
            xkv1_t = cpool.tile([1, KK], F32R, tag="xkv1", name="xkv1")
            nc.sync.dma_start(xkv1_t[:], xkv_d.ap()[DIM:DIM + 1, :])
            wq_t = cpool.tile([128, DT, HPC * DH], F32R, tag="wq", name="wq")
            nc.sync.dma_start(
                wq_t[:], wq_d.ap().rearrange("(t p) n -> p t n", p=128))
            xq_t = cpool.tile([128, DT, QPC], F32R, tag="xq", name="xq")
            xq_re = xq_d.ap().rearrange("(t p) n -> p t n", p=128)
            nc.sync.dma_start(xq_t[:, :, 0:512], xq_re[:, :, 0:512])
            nc.sync.dma_start(xkv_t[:, :, kdma[0][0]:kdma[0][1]],
                              xkv_re[:, :, kdma[0][0]:kdma[0][1]])
            for (c0, c1) in kdma[1:]:
                nc.sync.dma_start(xkv_t[:, :, c0:c1], xkv_re[:, :, c0:c1])
            nc.sync.dma_start(xq_t[:, :, 512:QPC], xq_re[:, :, 512:QPC])
            woa_t = cpool.tile([128, DIM], F32R, tag="woa", name="woa")
            nc.sync.dma_start(woa_t[:], wo_d.ap()[0:128, :])
            wob_t = cpool.tile([64, DIM], F32R, tag="wob", name="wob")
            nc.sync.dma_start(wob_t[:], wo_d.ap()[128:192, :])

            # ---- persistent intermediates ----
            qT = [cpool.tile([64, QPC], F32R, tag=f"qT{h}", name=f"qT{h}")
                  for h in range(HPC)]
            kT = [cpool.tile([64, KK], F32R, tag=f"kT{h}", name=f"kT{h}")
                  for h in range(HPC)]
            vv_t = cpool.tile([128, KT, HPC * ECH], F32R, tag="vv", name="vv")
            onorm_a = cpool.tile([128, QPC], F32R, tag="onorm_a", name="onorm_a")
            onorm_b = cpool.tile([64, QPC], F32R, tag="onorm_b", name="onorm_b")

            def q_proj(c0, c1):
                for (w0, w1, heads) in ((0, 128, (0, 1)), (128, 192, (2,))):
                    qps = ps_mm.tile([128, 512], F32, tag="mm", name="mm")
                    for dt in range(DT):
                        nc.tensor.matmul(
                            qps[0:w1 - w0, 0:c1 - c0],
                            wq_t[:, dt, w0:w1],
                            xq_t[:, dt, c0:c1],
                            start=(dt == 0), stop=(dt == DT - 1))
                    for i, h in enumerate(heads):
                        nc.vector.tensor_scalar_add(
                            qT[h][:, c0:c1],
                            qps[i * 64:(i + 1) * 64, 0:c1 - c0],
                            bqk_t[i * 64:(i + 1) * 64, h // 2:1 + (h // 2)])

            def v_proj(kt):
                vps = ps_mm.tile([128, 512], F32, tag="mm", name="mm")
                for dt in range(DT):
                    nc.tensor.matmul(
                        vps[:, 0:256],
                        xkv_t[:, dt, kt * 128:(kt + 1) * 128],
                        wv_t[:, dt, :],
                        start=(dt == 0), stop=False)
                nc.tensor.matmul(
                    vps[:, 0:256],
                    xkv1_t[:, kt * 128:(kt + 1) * 128],
                    wv1_t[:],
                    start=False, stop=True)
                nc.vector.tensor_scalar_mul(
                    vv_t[:, kt, :].rearrange("p (h z) -> p h z", z=ECH)[:, :, 0:DH],
                    vps[:, 0:HPC * DH].rearrange("p (h z) -> p h z", z=DH),
                    km_t[:, kt:kt + 1])
                for h in range(HPC):
                    nc.vector.tensor_copy(
                        vv_t[:, kt, h * ECH + DH:h * ECH + DH + 1],
                        km_t[:, kt:kt + 1])

            def k_proj(c0, c1):
                for (w0, w1, heads) in ((0, 128, (0, 1)), (128, 192, (2,))):
                    kps = ps_mm.tile([128, 512], F32, tag="mm", name="mm")
                    for dt in range(DT):
                        nc.tensor.matmul(
                            kps[0:w1 - w0, 0:c1 - c0],
                            wk_t[:, dt, w0:w1],
                            xkv_t[:, dt, c0:c1],
                            start=(dt == 0), stop=(dt == DT - 1))
                    for i, h in enumerate(heads):
                        nc.vector.tensor_scalar_add(
                            kT[h][:, c0:c1],
                            kps[i * 64:(i + 1) * 64, 0:c1 - c0],
                            bqk_t[i * 64:(i + 1) * 64, 2 + (h // 2):3 + (h // 2)])

            def attn_tile(qc, h, kt, ops_h, first, last):
                q0 = qc * 512
                sps = ps_mm.tile([128, 512], F32, tag="mm", name="mm")
                nc.tensor.matmul(
                    sps[:], kT[h][:, kt * 128:(kt + 1) * 128],
                    qT[h][:, q0:q0 + 512], start=True, stop=True)
                et = epool.tile([128, 512], F32R, tag="E", name="E")
                nc.scalar.activation(
                    et[:], sps[:], mybir.ActivationFunctionType.Exp, scale=SCALE)
                nc.tensor.matmul(
                    ops_h[:], vv_t[:, kt, h * ECH:(h + 1) * ECH], et[:],
                    start=first, stop=last)

            def normalize(qc, ops):
                q0 = qc * 512
                for h in range(HPC):
                    tln = npool.tile([1, 512], F32, tag="tln", name="tln")
                    nc.scalar.activation(
                        tln[:], ops[h][DH:DH + 1, :],
                        mybir.ActivationFunctionType.Ln, bias=epsc[:])
                    trc = npool.tile([1, 512], F32R, tag="trc", name="trc")
                    nc.scalar.activation(
                        trc[:], tln[:],
                        mybir.ActivationFunctionType.Exp, scale=negc[:])
                    bps = ps_bc.tile([64, 512], F32, tag="bc", name="bps")
                    nc.tensor.matmul(
                        bps[:], onesc[:].bitcast(F32R), trc[:],
                        start=True, stop=True)
                    bca = npool.tile([64, 512], F32, tag="bca", name="bca")
                    nc.any.tensor_copy(bca[:], bps[:])
                    dst = (onorm_a[h * 64:(h + 1) * 64, q0:q0 + 512] if h < 2
                           else onorm_b[:, q0:q0 + 512])
                    nc.vector.tensor_tensor(
                        dst, ops[h][0:DH, :], bca[:], op=mybir.AluOpType.mult)

            def out_proj(qc):
                q0 = qc * 512
                for qt in range(4):
                    t0 = q0 + qt * 128
                    for (o0, o1) in ((0, 384), (384, 768)):
                        pso = ps_mm.tile([128, 512], F32, tag="mm", name="mm")
                        nc.tensor.matmul(
                            pso[:, 0:o1 - o0], onorm_a[:, t0:t0 + 128],
                            woa_t[:, o0:o1], start=True, stop=False)
                        nc.tensor.matmul(
                            pso[:, 0:o1 - o0], onorm_b[:, t0:t0 + 128],
                            wob_t[:, o0:o1], start=False, stop=True)
                        osb = opool.tile([128, 384], F32, tag="osb", name="osb")
                        nc.any.tensor_copy(osb[:], pso[:, 0:o1 - o0])
                        nc.sync.dma_start(out_d.ap()[t0:t0 + 128, o0:o1], osb[:])

            # ---- phase schedule ----
            q_proj(0, 512)

            # qc0 attention streams behind the per-chunk K/V projections
            ops0 = [ps_op.tile([DH + 1, 512], F32, tag=f"op{h}", name=f"op{h}")
                    for h in range(HPC)]
            for (c0, c1) in kch:
                kts = range(c0 // 128, (c1 + 127) // 128)
                for kt in kts:
                    v_proj(kt)
                k_proj(c0, c1)
                for kt in kts:
                    for h in range(HPC):
                        attn_tile(0, h, kt, ops0[h], kt == 0, kt == KT - 1)

            normalize(0, ops0)
            q_proj(512, 1024)

            # qc1 attention: all inputs already resident; accumulators
            # reuse the qc0 slots, whose release normalize(0) just gated.
            ops1 = [ps_op.tile([DH + 1, 512], F32, tag=f"op{h}",
                               name=f"op{h}b") for h in range(HPC)]
            for kt in range(KT):
                for h in range(HPC):
                    attn_tile(1, h, kt, ops1[h], kt == 0, kt == KT - 1)

            out_proj(0)
            normalize(1, ops1)
            out_proj(1)

    nc.compile()
    return nc


def _get_nc(KT: int):
    if KT not in _CACHE:
        _CACHE[KT] = _build(KT)
    return _CACHE[KT]


def make_in_maps(x, last_score, Wq, bq, Wk, bk, Wv, bv, Wo, bo):
    """Host-side sharding: returns (in_maps, KT)."""
    x = np.ascontiguousarray(np.asarray(x, np.float32))
    last_score = np.asarray(last_score, np.float32)
    keep = np.nonzero(last_score > 0)[0]
    KK0 = len(keep)
    KT = max(1, (KK0 + 127) // 128)
    KK = KT * 128

    # gathered keys, transposed, plus a ones row for the V bias fold
    xkv = np.zeros((DIM + 1, KK), np.float32)
    xkv[0:DIM, 0:KK0] = x[keep].T
    xkv[DIM, :] = 1.0
    km = np.zeros((KK,), np.float32)
    km[0:KK0] = 1.0
    km2d = np.ascontiguousarray(km.reshape(KT, 128).T)  # [128, KT]

    xq = [np.ascontiguousarray(x[s * QPC:(s + 1) * QPC].T) for s in range(SG)]

    Wq = np.asarray(Wq, np.float32); Wk = np.asarray(Wk, np.float32)
    Wv = np.asarray(Wv, np.float32); Wo = np.asarray(Wo, np.float32)
    bq = np.asarray(bq, np.float32); bk = np.asarray(bk, np.float32)
    bv = np.asarray(bv, np.float32)

    in_maps = []
    for core in range(NCORES):
        g, s = core // SG, core % SG
        c0, c1 = g * HPC * DH, (g + 1) * HPC * DH
        wv_aug = np.zeros((DIM + 1, 256), np.float32)
        wv_aug[0:DIM, 0:HPC * DH] = Wv[:, c0:c1]
        wv_aug[DIM, 0:HPC * DH] = bv[c0:c1]
        bqk = np.zeros((128, 4), np.float32)
        bqk[:, 0] = bq[c0:c0 + 128]
        bqk[0:64, 1] = bq[c0 + 128:c1]
        bqk[:, 2] = bk[c0:c0 + 128]
        bqk[0:64, 3] = bk[c0 + 128:c1]
        in_maps.append({
            "xq": xq[s],
            "xkv": xkv,
            "wq": np.ascontiguousarray(Wq[:, c0:c1]),
            "wk": np.ascontiguousarray(Wk[:, c0:c1]),
            "wv": wv_aug,
            "wo": np.ascontiguousarray(Wo[c0:c1, :]),
            "bqk": bqk,
            "km": km2d,
        })
    return in_maps, KT


def combine_outputs(results, bo, last_score):
    out = np.zeros((N_CTX, DIM), np.float32)
    for core in range(NCORES):
        s = core % SG
        out[s * QPC:(s + 1) * QPC] += results[core]["out"]
    out += np.asarray(bo, np.float32)[None, :]
    return out, np.asarray(last_score, np.float32)


def kernel(x, last_score, Wq, bq, Wk, bk, Wv, bv, Wo, bo):
    in_maps, KT = make_in_maps(x, last_score, Wq, bq, Wk, bk, Wv, bv, Wo, bo)
    nc = _get_nc(KT)
    res = run_bass_kernel_spmd(nc, in_maps, list(range(NCORES)))
    return combine_outputs(res.results, bo, last_score)
